# revision 4
# baseline (speedup 1.0000x reference)
"""Trainium2 Bass kernel for the 4-modality attention-fusion module.

Computes, for full inputs mod0..mod3 [16384, 1024] f32 and W [1024, 1024] f32:
    scores_m = mod_m @ W.T                      (per modality)
    attn     = softmax over m of scores         (elementwise over [B, L])
    fused    = sum_m mod_m * attn_m
    scaler_b = 1 + #{m : sum_l mod_m[b, l] == 0}
    out      = fused * scaler[:, None]

Sharded data-parallel over 8 NeuronCores along the batch dim (2048 rows each),
W replicated. Design (v2 — PE runs ONLY the score matmuls):
  - W.T is built resident in SBUF in bf16 once, via XBAR DMA transposes
    (wt[p, j, k] = W[k, j*128 + p]);
  - per 128-patient tile, the f32 mod tiles are cast to bf16 on ACT; that
    same ACT op carries accum_out, producing the per-row sums (zero-modality
    detection) for free;
  - the bf16 mod tiles are transposed by the DMA XBAR engine (SBUF->SBUF,
    ~0.9us per [128,1024]) into the matmul stationary layout — the PE does
    no transposes and the ACT no PSUM evictions;
  - bf16 matmuls accumulate scores in PSUM (1 col/cycle, same rate f32r ran
    but without the PE transpose overhead);
  - softmax over the 4 modalities: exp on ACT straight out of PSUM;
    denominator adds on Pool (gpsimd); numerator products into separate
    tiles on DVE (no WAR stall against the Pool reads of e); reciprocal +
    final scaled multiply on DVE with the zero-modality rescale folded in;
  - per-segment emission order keeps next-tile casts ahead of this tile's
    exps in the ACT queue, and next-tile loads ahead of the XBAR transposes
    in the SP queue, so the PE never waits on stationaries.
"""

import sys

sys.path.insert(0, "/opt/trn_rl_repo")

from contextlib import ExitStack

import numpy as np

import concourse.bass as bass
import concourse.bacc as bacc
import concourse.mybir as mybir
import concourse.tile as tile
from concourse.bass_utils import run_bass_kernel_spmd

F32 = mybir.dt.float32
BF16 = mybir.dt.bfloat16
AF = mybir.ActivationFunctionType

N_CORES = 8
B_FULL = 16384
L = 1024
P = 128
B_SHARD = B_FULL // N_CORES          # 2048
NPT = B_SHARD // P                   # 16 patient tiles per core
NM = 4                               # modalities
NLC = L // P                         # 8 l-chunks (contraction)
NH = 2                               # k halves
KH = L // NH                         # 512

_CACHE: dict = {}


def _build(repeat: int = 1, *, elem: bool = True):
    nc = bacc.Bacc("TRN2", target_bir_lowering=False, debug=False)
    mods_d = [
        nc.dram_tensor(f"mod{m}", [B_SHARD, L], F32, kind="ExternalInput").ap()
        for m in range(NM)
    ]
    w_d = nc.dram_tensor("W", [L, L], F32, kind="ExternalInput").ap()
    out_d = nc.dram_tensor("out", [B_SHARD, L], F32, kind="ExternalOutput").ap()

    with tile.TileContext(nc) as tc, ExitStack() as ctx:
        wt_p = ctx.enter_context(tc.tile_pool(name="wt", bufs=1))
        wload_p = ctx.enter_context(tc.tile_pool(name="wload", bufs=2))
        mod_p = ctx.enter_context(tc.tile_pool(name="mod", bufs=4))
        modb_p = ctx.enter_context(tc.tile_pool(name="modb", bufs=2))
        modt_p = ctx.enter_context(tc.tile_pool(name="modt", bufs=2))
        e_p = ctx.enter_context(tc.tile_pool(name="e", bufs=4))
        n_p = ctx.enter_context(tc.tile_pool(name="n", bufs=2))
        rs_p = ctx.enter_context(tc.tile_pool(name="rs", bufs=4))
        tmp_p = ctx.enter_context(tc.tile_pool(name="tmp", bufs=2))
        out_p = ctx.enter_context(tc.tile_pool(name="outp", bufs=2))
        ps_q = ctx.enter_context(
            tc.tile_pool(name="ps_q", bufs=6, space=bass.MemorySpace.PSUM)
        )

        # ---- WT resident in SBUF (bf16): wt[p, j, k] = W[k, j*128 + p] ----
        wt = wt_p.tile([P, NLC, L], BF16, tag="wt")
        for kc in range(NLC):
            wk = wload_p.tile([P, L], F32, tag="wk")
            nc.sync.dma_start(wk[:], w_d[kc * P : (kc + 1) * P, :])
            wkb = wload_p.tile([P, L], BF16, tag="wkb")
            nc.scalar.copy(wkb[:], wk[:])
            nc.sync.dma_start_transpose(wt[:, :, kc * P : (kc + 1) * P], wkb[:])

        # ---------------- main loop, software-pipelined ----------------
        rep_cm = (
            tc.For_i(
                0,
                repeat,
                1,
                hint_engines=(
                    mybir.EngineType.PE,
                    mybir.EngineType.DVE,
                    mybir.EngineType.Activation,
                    mybir.EngineType.Pool,
                    mybir.EngineType.SP,
                ),
            )
            if repeat > 1
            else None
        )
        if rep_cm is not None:
            rep_cm.__enter__()

        def emit_load(p):
            row = slice(p * P, (p + 1) * P)
            mods = []
            for m in range(NM):
                mt = mod_p.tile([P, L], F32, tag=f"mod{m}")
                nc.sync.dma_start(mt[:], mods_d[m][row, :])
                mods.append(mt)
            return mods

        def emit_conv(p, mods):
            """f32 -> bf16 casts on ACT; accum_out rides along to produce the
            per-modality row sums (zero-modality detection) for free."""
            rsum = rs_p.tile([P, NM], F32, tag="rsum")
            modbs = []
            for m in range(NM):
                mb = modb_p.tile([P, L], BF16, tag=f"modb{m}")
                nc.scalar.activation(
                    mb[:], mods[m][:], AF.Copy, accum_out=rsum[:, m : m + 1]
                )
                modbs.append(mb)
            return modbs, rsum

        def emit_transp(p, modbs):
            modts = []
            for m in range(NM):
                mT = modt_p.tile([P, NLC, P], BF16, tag=f"modt{m}")
                nc.sync.dma_start_transpose(mT[:], modbs[m][:])
                modts.append(mT)
            return modts

        def emit_pe(p, modts):
            """Score matmuls + trailing exps for tile p."""
            es = {}
            for m in range(NM):
                sqs = []
                for _h in range(NH):
                    sq = ps_q.tile([P, KH], F32, tag="sq")
                    sqs.append(sq)
                for j in range(NLC):
                    for h in range(NH):
                        nc.tensor.matmul(
                            sqs[h][:],
                            modts[m][:, j, :],
                            wt[:, j, h * KH : (h + 1) * KH],
                            start=(j == 0),
                            stop=(j == NLC - 1),
                        )
                if elem:
                    for h in range(NH):
                        e = e_p.tile([P, KH], F32, tag=f"e{m}")
                        nc.scalar.activation(e[:], sqs[h][:], AF.Exp)
                        es[(m, h)] = e
            return es

        def emit_tail(state):
            """Softmax combine + output for tile p (lags one segment)."""
            p, mods, es, rsum = state
            row = slice(p * P, (p + 1) * P)
            zt = tmp_p.tile([P, NM], F32, tag="zt")
            zs = tmp_p.tile([P, 1], F32, tag="zs")
            nc.vector.tensor_scalar(
                out=zt[:],
                in0=rsum[:],
                scalar1=0.0,
                scalar2=None,
                op0=mybir.AluOpType.is_equal,
                op1=mybir.AluOpType.add,
                accum_out=zs[:],
            )
            scaler = tmp_p.tile([P, 1], F32, tag="scaler")
            nc.vector.tensor_scalar_add(scaler[:], zs[:], 1.0)
            ot = out_p.tile([P, L], F32, tag="ot")
            for h in range(NH):
                e0, e1, e2, e3 = (es[(m, h)] for m in range(NM))
                # denominator on Pool; reads e_m, no write-back into them
                d01 = tmp_p.tile([P, KH], F32, tag="d01")
                d23 = tmp_p.tile([P, KH], F32, tag="d23")
                nc.gpsimd.tensor_add(d01[:], e0[:], e1[:])
                nc.gpsimd.tensor_add(d23[:], e2[:], e3[:])
                nc.gpsimd.tensor_add(d01[:], d01[:], d23[:])
                # numerator into separate tiles on DVE (no WAR vs Pool reads)
                ns = []
                for m in range(NM):
                    nm_t = n_p.tile([P, KH], F32, tag=f"n{m}")
                    nc.vector.tensor_mul(
                        nm_t[:],
                        es[(m, h)][:],
                        mods[m][:, h * KH : (h + 1) * KH],
                    )
                    ns.append(nm_t)
                nc.vector.tensor_add(ns[0][:], ns[0][:], ns[1][:])
                nc.gpsimd.tensor_add(ns[2][:], ns[2][:], ns[3][:])
                nc.vector.reciprocal_approx_fast(out=d01[:], in_=d01[:])
                nc.vector.tensor_add(ns[0][:], ns[0][:], ns[2][:])
                # ot = (r * scaler) * num in one DVE op
                nc.vector.scalar_tensor_tensor(
                    out=ot[:, h * KH : (h + 1) * KH],
                    in0=d01[:],
                    scalar=scaler[:],
                    in1=ns[0][:],
                    op0=mybir.AluOpType.mult,
                    op1=mybir.AluOpType.mult,
                )
            nc.sync.dma_start(out_d[row, :], ot[:])

        # prologue
        loaded = {0: emit_load(0)}
        conv = {0: emit_conv(0, loaded[0])}
        transposed = {0: emit_transp(0, conv[0][0])}
        if NPT > 1:
            loaded[1] = emit_load(1)

        prev = None
        for p in range(NPT):
            if p + 2 < NPT:
                loaded[p + 2] = emit_load(p + 2)
            if p + 1 < NPT:
                conv[p + 1] = emit_conv(p + 1, loaded[p + 1])
                transposed[p + 1] = emit_transp(p + 1, conv[p + 1][0])
            es = emit_pe(p, transposed.pop(p))
            state = (p, loaded[p], es, conv[p][1])
            if prev is not None and elem:
                emit_tail(prev)
            prev = state
        if elem:
            emit_tail(prev)

        if rep_cm is not None:
            rep_cm.__exit__(None, None, None)

    nc.compile()
    return nc


def _get_nc(repeat: int = 1, **flags):
    key = ("nc", repeat, tuple(sorted(flags.items())))
    if key not in _CACHE:
        _CACHE[key] = _build(repeat, **flags)
    return _CACHE[key]


def _run(inputs, trace=False):
    nc = _get_nc()
    w = np.ascontiguousarray(np.asarray(inputs["W"], dtype=np.float32))
    in_maps = []
    for c in range(N_CORES):
        sl = slice(c * B_SHARD, (c + 1) * B_SHARD)
        im = {"W": w}
        for m in range(NM):
            im[f"mod{m}"] = np.ascontiguousarray(
                np.asarray(inputs[f"mod{m}"], dtype=np.float32)[sl]
            )
        in_maps.append(im)
    return run_bass_kernel_spmd(
        nc, in_maps, core_ids=list(range(N_CORES)), trace=trace
    )


def kernel(**inputs) -> np.ndarray:
    res = _run(inputs, trace=False)
    return np.concatenate(
        [res.results[c]["out"] for c in range(N_CORES)], axis=0
    ).astype(np.float32)


# revision 10
# speedup vs baseline: 1.6030x; 1.6030x over previous
"""Trainium2 Bass kernel for the 4-modality attention-fusion module.

Computes, for full inputs mod0..mod3 [16384, 1024] f32 and W [1024, 1024] f32:
    scores_m = mod_m @ W.T                      (per modality)
    attn     = softmax over m of scores         (elementwise over [B, L])
    fused    = sum_m mod_m * attn_m
    scaler_b = 1 + #{m : sum_l mod_m[b, l] == 0}
    out      = fused * scaler[:, None]

Sharded data-parallel over 8 NeuronCores along the batch dim (2048 rows each),
W replicated. Design (v2 — PE runs ONLY the score matmuls):
  - W.T is built resident in SBUF in bf16 once, via XBAR DMA transposes
    (wt[p, j, k] = W[k, j*128 + p]);
  - per 128-patient tile, the f32 mod tiles are cast to bf16 on ACT; that
    same ACT op carries accum_out, producing the per-row sums (zero-modality
    detection) for free;
  - the bf16 mod tiles are transposed by the DMA XBAR engine (SBUF->SBUF,
    ~0.9us per [128,1024]) into the matmul stationary layout — the PE does
    no transposes and the ACT no PSUM evictions;
  - bf16 matmuls accumulate scores in PSUM (1 col/cycle, same rate f32r ran
    but without the PE transpose overhead);
  - softmax over the 4 modalities: exp on ACT straight out of PSUM;
    denominator adds on Pool (gpsimd); numerator products into separate
    tiles on DVE (no WAR stall against the Pool reads of e); reciprocal +
    final scaled multiply on DVE with the zero-modality rescale folded in;
  - per-segment emission order keeps next-tile casts ahead of this tile's
    exps in the ACT queue, and next-tile loads ahead of the XBAR transposes
    in the SP queue, so the PE never waits on stationaries.
"""

import sys

sys.path.insert(0, "/opt/trn_rl_repo")

from contextlib import ExitStack

import numpy as np

import concourse.bass as bass
import concourse.bacc as bacc
import concourse.mybir as mybir
import concourse.tile as tile
from concourse.bass_utils import run_bass_kernel_spmd

F32 = mybir.dt.float32
BF16 = mybir.dt.bfloat16
AF = mybir.ActivationFunctionType

N_CORES = 8
B_FULL = 16384
L = 1024
P = 128
B_SHARD = B_FULL // N_CORES          # 2048
NPT = B_SHARD // P                   # 16 patient tiles per core
NM = 4                               # modalities
NLC = L // P                         # 8 l-chunks (contraction)
NH = 2                               # k halves
KH = L // NH                         # 512

_CACHE: dict = {}


def _build(repeat: int = 1, *, elem: bool = True, transp: bool = True):
    nc = bacc.Bacc("TRN2", target_bir_lowering=False, debug=False)
    mods_d = [
        nc.dram_tensor(f"mod{m}", [B_SHARD, L], F32, kind="ExternalInput").ap()
        for m in range(NM)
    ]
    w_d = nc.dram_tensor("W", [L, L], F32, kind="ExternalInput").ap()
    out_d = nc.dram_tensor("out", [B_SHARD, L], F32, kind="ExternalOutput").ap()

    with tile.TileContext(nc) as tc, ExitStack() as ctx:
        wt_p = ctx.enter_context(tc.tile_pool(name="wt", bufs=1))
        wload_p = ctx.enter_context(tc.tile_pool(name="wload", bufs=2))
        mod_p = ctx.enter_context(tc.tile_pool(name="mod", bufs=4))
        modb_p = ctx.enter_context(tc.tile_pool(name="modb", bufs=2))
        modt_p = ctx.enter_context(tc.tile_pool(name="modt", bufs=2))
        e_p = ctx.enter_context(tc.tile_pool(name="e", bufs=4))
        rs_p = ctx.enter_context(tc.tile_pool(name="rs", bufs=4))
        tmp_p = ctx.enter_context(tc.tile_pool(name="tmp", bufs=2))
        out_p = ctx.enter_context(tc.tile_pool(name="outp", bufs=2))
        ps_q = ctx.enter_context(
            tc.tile_pool(name="ps_q", bufs=6, space=bass.MemorySpace.PSUM)
        )

        # ---- WT resident in SBUF (bf16): wt[p, j, k] = W[k, j*128 + p] ----
        wt = wt_p.tile([P, NLC, L], BF16, tag="wt")
        for kc in range(NLC):
            wk = wload_p.tile([P, L], F32, tag="wk")
            nc.sync.dma_start(wk[:], w_d[kc * P : (kc + 1) * P, :])
            wkb = wload_p.tile([P, L], BF16, tag="wkb")
            nc.scalar.copy(wkb[:], wk[:])
            nc.sync.dma_start_transpose(wt[:, :, kc * P : (kc + 1) * P], wkb[:])

        # ---------------- main loop, software-pipelined ----------------
        rep_cm = (
            tc.For_i(
                0,
                repeat,
                1,
                hint_engines=(
                    mybir.EngineType.PE,
                    mybir.EngineType.DVE,
                    mybir.EngineType.Activation,
                    mybir.EngineType.Pool,
                    mybir.EngineType.SP,
                ),
            )
            if repeat > 1
            else None
        )
        if rep_cm is not None:
            rep_cm.__enter__()

        def emit_load(p):
            row = slice(p * P, (p + 1) * P)
            mods = []
            for m in range(NM):
                mt = mod_p.tile([P, L], F32, tag=f"mod{m}")
                nc.sync.dma_start(mt[:], mods_d[m][row, :])
                mods.append(mt)
            return mods

        def emit_conv(p, mods):
            """f32 -> bf16 casts on ACT; accum_out rides along to produce the
            per-modality row sums (zero-modality detection) for free."""
            rsum = rs_p.tile([P, NM], F32, tag="rsum")
            modbs = []
            for m in range(NM):
                mb = modb_p.tile([P, L], BF16, tag=f"modb{m}")
                nc.scalar.activation(
                    mb[:], mods[m][:], AF.Copy, accum_out=rsum[:, m : m + 1]
                )
                modbs.append(mb)
            return modbs, rsum

        def emit_transp(p, modbs):
            if not transp:
                return None
            modts = []
            for m in range(NM):
                mT = modt_p.tile([P, NLC, P], BF16, tag=f"modt{m}")
                nc.sync.dma_start_transpose(mT[:], modbs[m][:])
                modts.append(mT)
            return modts

        def emit_pe(p, modts):
            """Score matmuls + trailing exps for tile p."""
            es = {}
            for m in range(NM):
                sqs = []
                for _h in range(NH):
                    sq = ps_q.tile([P, KH], F32, tag="sq")
                    sqs.append(sq)
                for j in range(NLC):
                    lhsT = (
                        modts[m][:, j, :]
                        if modts is not None
                        else wt[:, j, 0:P]
                    )
                    for h in range(NH):
                        nc.tensor.matmul(
                            sqs[h][:],
                            lhsT,
                            wt[:, j, h * KH : (h + 1) * KH],
                            start=(j == 0),
                            stop=(j == NLC - 1),
                        )
                if elem:
                    for h in range(NH):
                        e = e_p.tile([P, KH], F32, tag=f"e{m}")
                        nc.scalar.activation(e[:], sqs[h][:], AF.Exp)
                        es[(m, h)] = e
            return es

        def emit_tail(state):
            """Softmax combine + output for tile p (lags one segment)."""
            p, mods, es, rsum = state
            row = slice(p * P, (p + 1) * P)
            zt = tmp_p.tile([P, NM], F32, tag="zt")
            zs = tmp_p.tile([P, 1], F32, tag="zs")
            nc.vector.tensor_scalar(
                out=zt[:],
                in0=rsum[:],
                scalar1=0.0,
                scalar2=None,
                op0=mybir.AluOpType.is_equal,
                op1=mybir.AluOpType.add,
                accum_out=zs[:],
            )
            scaler = tmp_p.tile([P, 1], F32, tag="scaler")
            nc.vector.tensor_scalar_add(scaler[:], zs[:], 1.0)
            ot = out_p.tile([P, L], F32, tag="ot")
            for h in range(NH):
                e0, e1, e2, e3 = (es[(m, h)] for m in range(NM))
                # all tail tensor ops on DVE: gpsimd (Pool) Add/Multiply run
                # far below roofline on HW (software Q7 implementation) and
                # became the critical path when loaded with the denominator
                d01 = tmp_p.tile([P, KH], F32, tag="d01")
                d23 = tmp_p.tile([P, KH], F32, tag="d23")
                nc.vector.tensor_add(d01[:], e0[:], e1[:])
                nc.vector.tensor_add(d23[:], e2[:], e3[:])
                nc.vector.tensor_add(d01[:], d01[:], d23[:])
                # numerator in place (same-engine queue, no WAR stall)
                for m in range(NM):
                    nc.vector.tensor_mul(
                        es[(m, h)][:],
                        es[(m, h)][:],
                        mods[m][:, h * KH : (h + 1) * KH],
                    )
                nc.vector.tensor_add(e0[:], e0[:], e1[:])
                nc.vector.tensor_add(e2[:], e2[:], e3[:])
                nc.vector.reciprocal_approx_fast(out=d01[:], in_=d01[:])
                nc.vector.tensor_add(e0[:], e0[:], e2[:])
                # ot = (r * scaler) * num in one DVE op
                nc.vector.scalar_tensor_tensor(
                    out=ot[:, h * KH : (h + 1) * KH],
                    in0=d01[:],
                    scalar=scaler[:],
                    in1=e0[:],
                    op0=mybir.AluOpType.mult,
                    op1=mybir.AluOpType.mult,
                )
            nc.sync.dma_start(out_d[row, :], ot[:])

        # prologue
        loaded = {0: emit_load(0)}
        conv = {0: emit_conv(0, loaded[0])}
        transposed = {0: emit_transp(0, conv[0][0])}
        if NPT > 1:
            loaded[1] = emit_load(1)

        prev = None
        for p in range(NPT):
            if p + 2 < NPT:
                loaded[p + 2] = emit_load(p + 2)
            if p + 1 < NPT:
                conv[p + 1] = emit_conv(p + 1, loaded[p + 1])
                transposed[p + 1] = emit_transp(p + 1, conv[p + 1][0])
            es = emit_pe(p, transposed.pop(p))
            state = (p, loaded[p], es, conv[p][1])
            if prev is not None and elem:
                emit_tail(prev)
            prev = state
        if elem:
            emit_tail(prev)

        if rep_cm is not None:
            rep_cm.__exit__(None, None, None)

    nc.compile()
    return nc


def _get_nc(repeat: int = 1, **flags):
    key = ("nc", repeat, tuple(sorted(flags.items())))
    if key not in _CACHE:
        _CACHE[key] = _build(repeat, **flags)
    return _CACHE[key]


def _run(inputs, trace=False):
    nc = _get_nc()
    w = np.ascontiguousarray(np.asarray(inputs["W"], dtype=np.float32))
    in_maps = []
    for c in range(N_CORES):
        sl = slice(c * B_SHARD, (c + 1) * B_SHARD)
        im = {"W": w}
        for m in range(NM):
            im[f"mod{m}"] = np.ascontiguousarray(
                np.asarray(inputs[f"mod{m}"], dtype=np.float32)[sl]
            )
        in_maps.append(im)
    return run_bass_kernel_spmd(
        nc, in_maps, core_ids=list(range(N_CORES)), trace=trace
    )


def kernel(**inputs) -> np.ndarray:
    res = _run(inputs, trace=False)
    return np.concatenate(
        [res.results[c]["out"] for c in range(N_CORES)], axis=0
    ).astype(np.float32)
